# revision 1
# baseline (speedup 1.0000x reference)
"""CTRGC Trainium2 kernel (v2).

Reference computation (per sample n):
  g     = Wg @ x[n] + bg                      [64, T=128, V=25]
  xm    = mean_t x[n]                         [64, 25]
  theta = Wth @ xm + bth ;  phi = Wph @ xm + bph        [16, 25]
  rel[i,a,b]  = tanh(theta[i,a] - phi[i,b])   [16, 25, 25]
  rel2        = Wr @ rel + br                 [64, 25, 25]
  A_dyn[c,a,b] = (A+PA)[a,b] + alpha*rel2[c,a,b]
  out[c,t,u]  = sum_v g[c,t,v] * A_dyn[c,u,v]

Sharding: data-parallel over N=128 samples across 8 cores (16 each),
processed on-core in pairs (2x64 channels = 128 partitions).

v2 dataflow (per pair):
  x uploaded v-major [.., (v,t)].
  g^T produced directly by matmul with x-slices stationary:
    out[t, (s,c)] per v -> gtt [t, (c',v32)]  (no PE transposes)
  one XBAR dma transpose: gtt -> gt4 [(q,v32), (grp, t)]
  A_dyn chain in u-major layout with a bias slot at v=25:
    advu [c, (u, v26)], col v=25 = bg[c]*sum_v A_dyn[c,u,v]
  step7: 128 row-tiled matmuls contract k=26 (25 v + ones row carries
    the g-bias term): out [t, (s,c,u)] -> bf16 -> DRAM [T, C*V].
  Host unshard: concat + transpose to [N, C, T, V] f32.
"""

import os
import sys

import numpy as np

sys.path.insert(0, "/opt/trn_rl_repo")

import concourse.bass as bass  # noqa: E402
import concourse.tile as tile  # noqa: E402
from concourse import bacc  # noqa: E402
from concourse import mybir  # noqa: E402
from concourse.bass_utils import run_bass_kernel_spmd  # noqa: E402

F32 = mybir.dt.float32
BF16 = mybir.dt.bfloat16

USE_POOL = os.environ.get("CTRGC_POOL", "1") == "1"
USE_XBAR = os.environ.get("CTRGC_XBAR", "1") == "1"
USE_GINT = os.environ.get("CTRGC_GINT", "1") == "1"

N, C_IN, C_OUT, C_INT, T, V = 128, 64, 64, 16, 128, 25
NCORES = 8
NSH = N // NCORES          # samples per core (16)
NPAIR = NSH // 2           # pairs per core (8)
TV = T * V                 # 3200
CU = C_OUT * V             # 1600
V1 = V + 1                 # 26: v plus the bias slot

_cache = {}


def _build_nc():
    nc = bacc.Bacc("TRN2", target_bir_lowering=False, debug=False)

    # x pre-permuted on host to v-major: [NSH, C_IN, (v,t)]
    xs_d = nc.dram_tensor("xs", [NSH, C_IN, TV], F32, kind="ExternalInput")
    # out stored [T, C*V] bf16 per sample; host transposes to [C, T, V] f32
    ys_d = nc.dram_tensor("ys", [NSH, T, CU], BF16, kind="ExternalOutput")

    ca_d = nc.dram_tensor("constsA", [128, 654], F32, kind="ExternalInput")
    cb_d = nc.dram_tensor("constsB", [128, 448], BF16, kind="ExternalInput")

    with tile.TileContext(nc) as tc:
        _body(nc, tc, xs_d, ys_d, ca_d, cb_d)
    nc.finalize()
    return nc


def _body(nc, tc, xs_d, ys_d, ca_d, cb_d):
    from contextlib import ExitStack
    ctx = ExitStack()
    with ctx:
        const = ctx.enter_context(tc.tile_pool(name="const", bufs=1))
        xin = ctx.enter_context(tc.tile_pool(name="xin", bufs=2))
        gttp = ctx.enter_context(tc.tile_pool(name="gtt", bufs=2))
        gt4p = ctx.enter_context(tc.tile_pool(name="gt4", bufs=2))
        adp = ctx.enter_context(tc.tile_pool(name="ad", bufs=2))
        outp = ctx.enter_context(tc.tile_pool(name="outs", bufs=2))
        smallp = ctx.enter_context(tc.tile_pool(name="small", bufs=3))

        psg = ctx.enter_context(tc.tile_pool(name="psg", bufs=2, space="PSUM"))
        ps7 = ctx.enter_context(tc.tile_pool(name="ps7", bufs=1, space="PSUM"))
        psaux = ctx.enter_context(tc.tile_pool(name="psaux", bufs=1,
                                               space="PSUM"))
        psadt = ctx.enter_context(tc.tile_pool(name="psadt", bufs=1,
                                               space="PSUM"))

        cA = const.tile([128, 654], F32)
        nc.sync.dma_start(cA[:], ca_d[:])
        cB = const.tile([128, 448], BF16)
        nc.sync.dma_start(cB[:], cb_d[:])

        bgp = cA[:, 0:1]
        bthp = cA[0:32, 1:2]
        bphp = cA[0:32, 2:3]
        strepA = cA[:, 4:654]          # [c, (u,v26)] S[u,v] + alpha*br[c]
        wgT = cB[:, 0:128]
        wthT = cB[:, 128:160]
        wphT = cB[:, 160:192]
        wrTa = cB[0:32, 192:320]
        tident = cB[:, 320:448]

        # copy-engine rotation, DVE-heavy (gpsimd cannot read PSUM and
        # ACT is ~2x slower per element)
        def _copy(i, out_ap, in_ap):
            if i % 3 == 2:
                nc.scalar.copy(out_ap, in_ap)
            else:
                nc.vector.tensor_copy(out_ap, in_ap)

        for p in range(NPAIR):
            # ---- load x pair: [128 (2s x 64cin), 3200 (v,t)] ----
            xp = xin.tile([128, TV], BF16, tag="xp")
            nc.gpsimd.dma_start(
                xp[:], xs_d[2 * p:2 * p + 2].rearrange("n c f -> (n c) f"))

            # ---- g^T direct: per v, out[t, (s,c)] ; gtt [t, (c',v32)] ----
            gtt = gttp.tile([128, 128 * 32], BF16, tag="gtt")
            gtt_v = gtt[:].rearrange("p (c v) -> p v c", v=32)
            # bias slot v=25 <- 1.0 (ones row for step7 k=26); v=26.. <- 0
            nc.gpsimd.memset(gtt_v[:, V:V + 1, :], 1.0)
            nc.gpsimd.memset(gtt_v[:, V + 1:32, :], 0.0)
            ci = 0
            for v0, vn in _chunks(V, 4):
                gps = psg.tile([128, 512], F32, tag="gps")
                if USE_GINT:
                    # interleave (c',v) in PSUM so the copy dst has 4-elem
                    # runs instead of single-elem runs
                    gps_v = gps[:, 0:vn * 128].rearrange(
                        "p (c v) -> p v c", v=vn)
                    for vi in range(vn):
                        nc.tensor.matmul(
                            gps_v[:, vi, :],
                            xp[:, (v0 + vi) * T:(v0 + vi + 1) * T], wgT,
                            start=True, stop=True)
                    _copy(ci, gtt[:].rearrange(
                              "p (c v) -> p c v", v=32)[:, :, v0:v0 + vn],
                          gps[:, 0:vn * 128].rearrange(
                              "p (c v) -> p c v", v=vn))
                else:
                    for vi in range(vn):
                        v = v0 + vi
                        nc.tensor.matmul(
                            gps[:, vi * 128:vi * 128 + 128],
                            xp[:, v * T:(v + 1) * T], wgT,
                            start=True, stop=True)
                    _copy(ci, gtt_v[:, v0:v0 + vn, :],
                          gps[:, 0:vn * 128].rearrange(
                              "p (v c) -> p v c", c=128))
                ci += 1

            # ---- xsum over t (v-major: unit stride) -> theta/phi ----
            xsum = smallp.tile([128, V], F32, tag="xsum")
            nc.vector.tensor_reduce(
                out=xsum[:], in_=xp[:].rearrange("p (v t) -> p v t", v=V),
                axis=mybir.AxisListType.X, op=mybir.AluOpType.add)
            xsumb = smallp.tile([128, V], BF16, tag="xsumb")
            (nc.gpsimd if USE_POOL else nc.vector).tensor_copy(
                xsumb[:], xsum[:])

            thps = psaux.tile([128, 512], F32, tag="auxps")
            nc.tensor.matmul(thps[0:32, 0:V], wthT, xsumb[:],
                             start=True, stop=True)
            th = smallp.tile([32, V], F32, tag="th")
            nc.scalar.activation(th[:], thps[0:32, 0:V],
                                 mybir.ActivationFunctionType.Identity,
                                 bias=bthp)

            phps = psaux.tile([128, 512], F32, tag="auxps")
            nc.tensor.matmul(phps[0:32, 0:V], wphT, xsumb[:],
                             start=True, stop=True)
            ph = smallp.tile([32, V], F32, tag="ph")
            nc.scalar.activation(ph[:], phps[0:32, 0:V],
                                 mybir.ActivationFunctionType.Identity,
                                 bias=bphp)

            # ---- rel[i, (u,v26)] = tanh(th[i,u] - ph[i,v]) ----
            reld = smallp.tile([32, V * V1], F32, tag="reld")
            r3 = reld[:].rearrange("p (u v) -> p u v", v=V1)
            nc.gpsimd.memset(r3[:, :, V:V1], 0.0)  # bias slot: tanh(0)=0
            th_b = th[:].rearrange("p (u o) -> p u o", o=1).broadcast_to(
                [32, V, V])
            ph_b = ph[:].rearrange("p (o v) -> p o v", o=1).broadcast_to(
                [32, V, V])
            (nc.gpsimd if USE_POOL else nc.vector).tensor_tensor(
                out=r3[:, :, 0:V], in0=th_b, in1=ph_b,
                op=mybir.AluOpType.subtract)
            relt = smallp.tile([32, V * V1], BF16, tag="relt")
            nc.scalar.activation(
                relt[:], reld[:], mybir.ActivationFunctionType.Tanh)

            # ---- rel2 -> advu [c, (u, v26)] = A_dyn[c,u,v] (bf16) ----
            # strepA already carries S[u,v] + alpha*br[c] (host-folded)
            advu = adp.tile([128, V * V1], BF16, tag="advu")
            for c0, cn in _chunks(V * V1, 512):
                r2ps = psaux.tile([128, 512], F32, tag="auxps")
                nc.tensor.matmul(r2ps[:, 0:cn], wrTa, relt[:, c0:c0 + cn],
                                 start=True, stop=True)
                nc.vector.tensor_tensor(
                    out=advu[:, c0:c0 + cn], in0=r2ps[:, 0:cn],
                    in1=strepA[:, c0:c0 + cn], op=mybir.AluOpType.add)

            # bias slot v=25: bg[c] * sum_v A_dyn[c,u,v]
            ad3 = advu[:].rearrange("p (u v) -> p u v", v=V1)
            sumA = smallp.tile([128, V], F32, tag="sumA")
            nc.vector.tensor_reduce(
                out=sumA[:], in_=ad3[:, :, 0:V],
                axis=mybir.AxisListType.X, op=mybir.AluOpType.add)
            (nc.gpsimd if USE_POOL else nc.vector).tensor_scalar(
                out=ad3[:, :, V:V1],
                in0=sumA[:].rearrange("p (u o) -> p u o", o=1),
                scalar1=bgp, scalar2=None, op0=mybir.AluOpType.mult)

            # ---- adtt [v26(+strips), (u,c)] via PE transposes ----
            # (u,c)-major keeps the PSUM->SBUF copies fully contiguous;
            # step7 reads its per-channel [26, 25u] rhs strided instead.
            adtt = adp.tile([128, 128 * V], BF16, tag="adtt")
            for u0, un in _chunks(V, 8):
                atps = psadt.tile([32, 1024], BF16, tag="atps")
                for ui in range(un):
                    nc.tensor.transpose(
                        atps[0:V1, ui * 128:ui * 128 + 128],
                        advu[:, (u0 + ui) * V1:(u0 + ui + 1) * V1], tident)
                _copy(ci, adtt[0:V1, u0 * 128:(u0 + un) * 128],
                      atps[0:V1, 0:un * 128])
                ci += 1
            for q in range(1, 4):
                nc.sync.dma_start(adtt[32 * q:32 * q + V1, :],
                                  adtt[0:V1, :])

            # ---- XBAR: gtt [t, (c',v32)] -> gt4 [(q,v32), (grp, t)] ----
            gt4 = gt4p.tile([128, 32 * 128], BF16, tag="gt4")
            if USE_XBAR:
                nc.sync.dma_start_transpose(
                    out=gt4[:].rearrange("p (g t) -> p g t", t=128),
                    in_=gtt[:])
            else:
                for b0, bn in _chunks(32, 4):
                    t3ps = psg.tile([128, 512], BF16, tag="gps")
                    for si in range(bn):
                        nc.tensor.transpose(
                            t3ps[:, si * 128:si * 128 + 128],
                            gtt[:, (b0 + si) * 128:(b0 + si) * 128 + 128],
                            tident)
                    _copy(ci, gt4[:, b0 * 128:(b0 + bn) * 128],
                          t3ps[:, 0:bn * 128])
                    ci += 1

            # ---- step7: row-tiled matmuls, k=26 (v + bias row) ----
            # one PSUM tile per PE row-band q (mixing tile_positions in a
            # single PSUM tile is not safe)
            out_sb = outp.tile([128, 2 * CU], BF16, tag="outsb")
            for b0 in range(0, 32, 16):
                p7q = [ps7.tile([128, 400], F32, tag=f"p7{q}",
                                name=f"p7{q}_{p}_{b0}") for q in range(4)]
                for gi in range(16):
                    g4i = b0 + gi
                    for q in range(4):
                        c = 4 * g4i + q
                        nc.tensor.matmul(
                            p7q[q][:, gi * V:(gi + 1) * V],
                            gt4[32 * q:32 * q + V1,
                                g4i * 128:g4i * 128 + 128],
                            adtt[32 * q:32 * q + V1, :].rearrange(
                                "p (u c) -> p c u", c=128)[:, c, :],
                            start=True, stop=True,
                            tile_position=(32 * q, 0))
                dst = out_sb[:, b0 * 100:(b0 + 16) * 100].rearrange(
                    "p (g q u) -> p q g u", q=4, u=V)
                for q in range(4):
                    _copy(ci, dst[:, q, :, :],
                          p7q[q][:, 0:400].rearrange("p (g u) -> p g u", u=V))
                    ci += 1
                del p7q

            # ---- store: per sample, contiguous [t, (c,u)] bf16 rows ----
            for s in range(2):
                nc.sync.dma_start(ys_d[2 * p + s],
                                  out_sb[:, s * CU:(s + 1) * CU])


def _chunks(total, step):
    out = []
    s = 0
    while s < total:
        out.append((s, min(step, total - s)))
        s += step
    return out


def _host_params(A, PA, alpha, Wg, bg, Wth, bth, Wph, bph, Wr, br):
    f = np.float32
    al = np.float32(alpha[0])
    wgT = np.zeros((128, 128), f)
    wgT[:64, :64] = Wg.T
    wgT[64:, 64:] = Wg.T
    # x_mean: fold 1/T into Wth/Wph lhsT
    wthT = np.zeros((128, 32), f)
    wthT[:64, :16] = Wth.T / T
    wthT[64:, 16:] = Wth.T / T
    wphT = np.zeros((128, 32), f)
    wphT[:64, :16] = Wph.T / T
    wphT[64:, 16:] = Wph.T / T
    wrTa = np.zeros((32, 128), f)
    wrTa[:16, :64] = al * Wr.T
    wrTa[16:, 64:] = al * Wr.T
    bgp = np.concatenate([bg, bg]).astype(f).reshape(128, 1)
    bthp = np.concatenate([bth, bth]).astype(f).reshape(32, 1)
    bphp = np.concatenate([bph, bph]).astype(f).reshape(32, 1)
    abrp = (al * np.concatenate([br, br])).astype(f).reshape(128, 1)
    S = (A + PA).astype(f)
    # strepA[c, u*26+v] = S[u,v] + alpha*br[c] for v<25; slot v=25 -> 0
    sU = np.zeros((V, V1), f)
    sU[:, :V] = S
    strepA = np.tile(sU.reshape(1, -1), (128, 1)).astype(f)
    mask = (np.arange(V * V1) % V1 < V).astype(f).reshape(1, -1)
    strepA = strepA + abrp @ mask
    cA = np.zeros((128, 654), f)
    cA[:, 0:1] = bgp
    cA[0:32, 1:2] = bthp
    cA[0:32, 2:3] = bphp
    cA[:, 4:654] = strepA
    bf16 = __import__("ml_dtypes").bfloat16
    cB = np.zeros((128, 448), f)
    cB[:, 0:128] = wgT
    cB[:, 128:160] = wthT
    cB[:, 160:192] = wphT
    cB[0:32, 192:320] = wrTa
    cB[:, 320:448] = np.eye(128, dtype=f)
    return dict(constsA=cA, constsB=cB.astype(bf16))


def kernel(**inputs):
    x = np.asarray(inputs["x"], np.float32)
    params = _host_params(
        np.asarray(inputs["A"], np.float32), np.asarray(inputs["PA"], np.float32),
        np.asarray(inputs["alpha"], np.float32), np.asarray(inputs["Wg"], np.float32),
        np.asarray(inputs["bg"], np.float32), np.asarray(inputs["Wth"], np.float32),
        np.asarray(inputs["bth"], np.float32), np.asarray(inputs["Wph"], np.float32),
        np.asarray(inputs["bph"], np.float32), np.asarray(inputs["Wr"], np.float32),
        np.asarray(inputs["br"], np.float32))

    if "nc" not in _cache:
        _cache["nc"] = _build_nc()
    nc = _cache["nc"]

    # upload x v-major: [NSH, C_IN, (v,t)]
    xv = np.ascontiguousarray(x.transpose(0, 1, 3, 2)).reshape(N, C_IN, TV)
    in_maps = []
    for i in range(NCORES):
        m = {"xs": xv[i * NSH:(i + 1) * NSH]}
        m.update(params)
        in_maps.append(m)

    res = run_bass_kernel_spmd(nc, in_maps, list(range(NCORES)),
                               **_cache.get("run_kwargs", {}))
    # device emits [NSH, T, C*V] bf16; unshard + fix layout to [N, C, T, V]
    out = np.concatenate([np.asarray(res.results[i]["ys"]) for i in range(NCORES)],
                         axis=0)
    out = out.reshape(N, T, C_OUT, V).transpose(0, 2, 1, 3)
    _cache["last_results"] = res
    return np.ascontiguousarray(out, dtype=np.float32)


if __name__ == "__main__":
    nc = _build_nc()
    print("build ok")



# revision 3
# speedup vs baseline: 1.0372x; 1.0372x over previous
"""CTRGC Trainium2 kernel (v3).

Reference computation (per sample n):
  g     = Wg @ x[n] + bg                      [64, T=128, V=25]
  xm    = mean_t x[n]                         [64, 25]
  theta = Wth @ xm + bth ;  phi = Wph @ xm + bph        [16, 25]
  rel[i,a,b]  = tanh(theta[i,a] - phi[i,b])   [16, 25, 25]
  rel2        = Wr @ rel + br                 [64, 25, 25]
  A_dyn[c,a,b] = (A+PA)[a,b] + alpha*rel2[c,a,b]
  out[c,t,u]  = sum_v g[c,t,v] * A_dyn[c,u,v]

Sharding: data-parallel over N=128 samples across 8 cores (16 each),
processed on-core in pairs (2x64 channels = 128 partitions).

v3 dataflow (per pair), instruction-count-driven (v2 was bound by
1456 LDWEIGHTS+MATMUL pairs and pipeline stalls):
  g^T via 25 x-stationary matmuls -> gtt [t, (c',v32)]  (as v2)
  one XBAR dma transpose: gtt -> gt4 [(q,v32), (b, t)]  (channel 4b+q)
  A_dyn chain with channel-permuted advu (partition p = (c%4)*32+c/4)
  4 scatter-DMAs build a block-diag moving operand BD [128, 3200]:
    BD[cb*32+v, cb*800+u*32+b] = A_dynT[v,u] of channel 4b+cb
    (two persistent buffers, off-block zeros written once at start)
  step7: 32 matmuls/pair (4 channels each): stationary = gt4 block
    [128,128], moving = BD view [128, (cb,u)=100] -> out [t, (cb,u)]
  Constant-region memsets hoisted out of the pair loop; stores and
  scatter split across the two HWDGE queues so the XBAR never blocks.
  Emission is software-pipelined: step7(p-1) is emitted after
  stage-A(p) so the tensor queue has no intra-pair bubbles.
"""

import os
import sys

import numpy as np

sys.path.insert(0, "/opt/trn_rl_repo")

import concourse.bass as bass  # noqa: E402
import concourse.tile as tile  # noqa: E402
from concourse import bacc  # noqa: E402
from concourse import mybir  # noqa: E402
from concourse.bass_utils import run_bass_kernel_spmd  # noqa: E402

F32 = mybir.dt.float32
BF16 = mybir.dt.bfloat16

N, C_IN, C_OUT, C_INT, T, V = 128, 64, 64, 16, 128, 25
NCORES = 8
NSH = N // NCORES          # samples per core (16)
NPAIR = NSH // 2           # pairs per core (8)
TV = T * V                 # 3200
CU = C_OUT * V             # 1600
V1 = V + 1                 # 26: v plus the bias slot

_cache = {}


def _build_nc():
    nc = bacc.Bacc("TRN2", target_bir_lowering=False, debug=False)

    # x pre-permuted on host to v-major: [NSH, C_IN, (v,t)]
    xs_d = nc.dram_tensor("xs", [NSH, C_IN, TV], F32, kind="ExternalInput")
    # out stored [T, (b,cb,u)] bf16 per sample; host fixes layout
    ys_d = nc.dram_tensor("ys", [NSH, T, CU], BF16, kind="ExternalOutput")

    ca_d = nc.dram_tensor("constsA", [128, 654], F32, kind="ExternalInput")
    cb_d = nc.dram_tensor("constsB", [128, 448], BF16, kind="ExternalInput")

    with tile.TileContext(nc) as tc:
        _body(nc, tc, xs_d, ys_d, ca_d, cb_d)
    nc.finalize()
    return nc


def _body(nc, tc, xs_d, ys_d, ca_d, cb_d):
    from contextlib import ExitStack
    ctx = ExitStack()
    with ctx:
        const = ctx.enter_context(tc.tile_pool(name="const", bufs=1))
        xin = ctx.enter_context(tc.tile_pool(name="xin", bufs=2))
        gttp = ctx.enter_context(tc.tile_pool(name="gtt", bufs=2))
        gt4p = ctx.enter_context(tc.tile_pool(name="gt4", bufs=3))
        adp = ctx.enter_context(tc.tile_pool(name="ad", bufs=2))
        outp = ctx.enter_context(tc.tile_pool(name="outs", bufs=2))
        smallp = ctx.enter_context(tc.tile_pool(name="small", bufs=3))

        psg = ctx.enter_context(tc.tile_pool(name="psg", bufs=2, space="PSUM"))
        ps7 = ctx.enter_context(tc.tile_pool(name="ps7", bufs=2, space="PSUM"))
        psaux = ctx.enter_context(tc.tile_pool(name="psaux", bufs=1,
                                               space="PSUM"))
        psadt = ctx.enter_context(tc.tile_pool(name="psadt", bufs=1,
                                               space="PSUM"))

        cA = const.tile([128, 654], F32)
        nc.sync.dma_start(cA[:], ca_d[:])
        cB = const.tile([128, 448], BF16)
        nc.sync.dma_start(cB[:], cb_d[:])

        bgp = cA[:, 0:1]               # permuted
        bthp = cA[0:32, 1:2]
        bphp = cA[0:32, 2:3]
        strepA = cA[:, 4:654]          # permuted rows: S[u,v] + a*br[c]
        wgT = cB[:, 0:128]
        wthT = cB[:, 128:160]
        wphT = cB[:, 160:192]
        wrTa = cB[0:32, 192:320]       # permuted cols
        tident = cB[:, 320:448]

        # two persistent block-diag operand buffers; off-block zeros are
        # written once here and never dirtied (scatter writes only the
        # in-block regions each pair)
        bd0 = const.tile([128, 3200], BF16, name="bd0")
        bd1 = const.tile([128, 3200], BF16, name="bd1")
        nc.gpsimd.memset(bd0[:], 0.0)
        nc.gpsimd.memset(bd1[:], 0.0)
        bds = [bd0, bd1]

        # copy-engine rotation, DVE-heavy (gpsimd cannot read PSUM and
        # ACT is ~2x slower per element)
        cictr = [0]

        def _copy(out_ap, in_ap):
            i = cictr[0]
            cictr[0] += 1
            if i % 3 == 2:
                nc.scalar.copy(out_ap, in_ap)
            else:
                nc.vector.tensor_copy(out_ap, in_ap)

        def stage_a(p):
            # ---- load x pair: [128 (2s x 64cin), 3200 (v,t)] ----
            xp = xin.tile([128, TV], BF16, tag="xp")
            nc.gpsimd.dma_start(
                xp[:], xs_d[2 * p:2 * p + 2].rearrange("n c f -> (n c) f"))

            # ---- g^T direct: per v, out[t, (s,c)] ; gtt [t, (c',v32)] ----
            gtt = gttp.tile([128, 128 * 32], BF16, tag="gtt")
            gtt_v = gtt[:].rearrange("p (c v) -> p v c", v=32)
            if p < 2:
                # constant slots, written once per physical buffer:
                # v=25 <- 1.0 (ones row for the k=26 bias term); v>=26 <- 0
                nc.gpsimd.memset(gtt_v[:, V:V + 1, :], 1.0)
                nc.gpsimd.memset(gtt_v[:, V + 1:32, :], 0.0)
            for v0, vn in _chunks(V, 4):
                gps = psg.tile([128, 512], F32, tag="gps")
                # interleave (c',v) in PSUM so the copy dst has 4-elem
                # runs instead of single-elem runs
                gps_v = gps[:, 0:vn * 128].rearrange(
                    "p (c v) -> p v c", v=vn)
                for vi in range(vn):
                    nc.tensor.matmul(
                        gps_v[:, vi, :],
                        xp[:, (v0 + vi) * T:(v0 + vi + 1) * T], wgT,
                        start=True, stop=True)
                _copy(gtt[:].rearrange(
                          "p (c v) -> p c v", v=32)[:, :, v0:v0 + vn],
                      gps[:, 0:vn * 128].rearrange(
                          "p (c v) -> p c v", v=vn))

            # ---- XBAR: gtt [t, (c',v32)] -> gt4 [(q,v32), (b, t)] ----
            gt4 = gt4p.tile([128, 32 * 128], BF16, tag="gt4")
            nc.sync.dma_start_transpose(
                out=gt4[:].rearrange("p (g t) -> p g t", t=128),
                in_=gtt[:])

            # ---- xsum over t (v-major: unit stride) -> theta/phi ----
            xsum = smallp.tile([128, V], F32, tag="xsum")
            nc.vector.tensor_reduce(
                out=xsum[:], in_=xp[:].rearrange("p (v t) -> p v t", v=V),
                axis=mybir.AxisListType.X, op=mybir.AluOpType.add)
            xsumb = smallp.tile([128, V], BF16, tag="xsumb")
            nc.gpsimd.tensor_copy(xsumb[:], xsum[:])

            thps = psaux.tile([128, 512], F32, tag="auxps")
            nc.tensor.matmul(thps[0:32, 0:V], wthT, xsumb[:],
                             start=True, stop=True)
            th = smallp.tile([32, V], F32, tag="th")
            nc.scalar.activation(th[:], thps[0:32, 0:V],
                                 mybir.ActivationFunctionType.Identity,
                                 bias=bthp)

            phps = psaux.tile([128, 512], F32, tag="auxps")
            nc.tensor.matmul(phps[0:32, 0:V], wphT, xsumb[:],
                             start=True, stop=True)
            ph = smallp.tile([32, V], F32, tag="ph")
            nc.scalar.activation(ph[:], phps[0:32, 0:V],
                                 mybir.ActivationFunctionType.Identity,
                                 bias=bphp)

            # ---- rel[i, (u,v26)] = tanh(th[i,u] - ph[i,v]) ----
            # (bias slot v=25 is garbage here; it is overwritten in advu)
            reld = smallp.tile([32, V * V1], F32, tag="reld")
            r3 = reld[:].rearrange("p (u v) -> p u v", v=V1)
            th_b = th[:].rearrange("p (u o) -> p u o", o=1).broadcast_to(
                [32, V, V])
            ph_b = ph[:].rearrange("p (o v) -> p o v", o=1).broadcast_to(
                [32, V, V])
            nc.gpsimd.tensor_tensor(
                out=r3[:, :, 0:V], in0=th_b, in1=ph_b,
                op=mybir.AluOpType.subtract)
            relt = smallp.tile([32, V * V1], BF16, tag="relt")
            nc.scalar.activation(
                relt[:], reld[:], mybir.ActivationFunctionType.Tanh)

            # ---- rel2 -> advu [p, (u, v26)] = A_dyn[c,u,v] (bf16) ----
            # partition p = (c%4)*32 + c//4 (wrTa cols + strepA rows are
            # host-permuted to match); strepA carries S[u,v] + alpha*br[c]
            advu = adp.tile([128, V * V1], BF16, tag="advu")
            for c0, cn in _chunks(V * V1, 512):
                r2ps = psaux.tile([128, 512], F32, tag="auxps")
                nc.tensor.matmul(r2ps[:, 0:cn], wrTa, relt[:, c0:c0 + cn],
                                 start=True, stop=True)
                nc.vector.tensor_tensor(
                    out=advu[:, c0:c0 + cn], in0=r2ps[:, 0:cn],
                    in1=strepA[:, c0:c0 + cn], op=mybir.AluOpType.add)

            # bias slot v=25: bg[c] * sum_v A_dyn[c,u,v]
            ad3 = advu[:].rearrange("p (u v) -> p u v", v=V1)
            sumA = smallp.tile([128, V], F32, tag="sumA")
            nc.vector.tensor_reduce(
                out=sumA[:], in_=ad3[:, :, 0:V],
                axis=mybir.AxisListType.X, op=mybir.AluOpType.add)
            nc.gpsimd.tensor_scalar(
                out=ad3[:, :, V:V1],
                in0=sumA[:].rearrange("p (u o) -> p u o", o=1),
                scalar1=bgp, scalar2=None, op0=mybir.AluOpType.mult)

            # ---- adtt [v26, (u, p)] via PE transposes ----
            adtt = adp.tile([128, 128 * V], BF16, tag="adtt")
            for u0, un in _chunks(V, 8):
                atps = psadt.tile([32, 1024], BF16, tag="atps")
                for ui in range(un):
                    nc.tensor.transpose(
                        atps[0:V1, ui * 128:ui * 128 + 128],
                        advu[:, (u0 + ui) * V1:(u0 + ui + 1) * V1], tident)
                _copy(adtt[0:V1, u0 * 128:(u0 + un) * 128],
                      atps[0:V1, 0:un * 128])

            # ---- scatter: adtt -> BD block-diag (partition shift via
            # DMA); split across the two HWDGE queues ----
            bd = bds[p % 2]
            a3 = adtt[0:V1, :].rearrange("p (u c) -> p u c", c=128)
            for cb in range(4):
                src = a3[:, :, cb * 32:(cb + 1) * 32]
                dst = bd[cb * 32:cb * 32 + V1,
                         cb * 800:cb * 800 + 800].rearrange(
                             "p (u b) -> p u b", b=32)
                (nc.sync if cb < 2 else nc.scalar).dma_start(dst, src)
            return gt4

        def stage_b(p, gt4):
            # ---- step7: 32 block-diag matmuls, 4 channels each ----
            bd = bds[p % 2]
            bd_v = bd[:].rearrange("p (c u b) -> p b c u", c=4, u=V, b=32)
            out_sb = outp.tile([128, 2 * CU], BF16, tag="outsb")
            for grp in range(8):
                p7 = ps7.tile([128, 400], F32, tag="p7")
                for j in range(4):
                    b = 4 * grp + j
                    nc.tensor.matmul(
                        p7[:, j * 100:(j + 1) * 100],
                        gt4[:, b * 128:(b + 1) * 128],
                        bd_v[:, b, :, :],
                        start=True, stop=True)
                _copy(out_sb[:, grp * 400:(grp + 1) * 400], p7[:, 0:400])

            # ---- store: per sample, contiguous [t, (b,cb,u)] rows ----
            for s in range(2):
                (nc.scalar if s == 0 else nc.sync).dma_start(
                    ys_d[2 * p + s], out_sb[:, s * CU:(s + 1) * CU])

        prev = None
        for p in range(NPAIR):
            gt4 = stage_a(p)
            if prev is not None:
                stage_b(p - 1, prev)
            prev = gt4
        stage_b(NPAIR - 1, prev)


def _chunks(total, step):
    out = []
    s = 0
    while s < total:
        out.append((s, min(step, total - s)))
        s += step
    return out


def _host_params(A, PA, alpha, Wg, bg, Wth, bth, Wph, bph, Wr, br):
    f = np.float32
    al = np.float32(alpha[0])
    # channel permutation: advu partition p holds channel inv(p)
    inv = (np.arange(128) % 32) * 4 + np.arange(128) // 32
    wgT = np.zeros((128, 128), f)
    wgT[:64, :64] = Wg.T
    wgT[64:, 64:] = Wg.T
    # x_mean: fold 1/T into Wth/Wph lhsT
    wthT = np.zeros((128, 32), f)
    wthT[:64, :16] = Wth.T / T
    wthT[64:, 16:] = Wth.T / T
    wphT = np.zeros((128, 32), f)
    wphT[:64, :16] = Wph.T / T
    wphT[64:, 16:] = Wph.T / T
    wrTa = np.zeros((32, 128), f)
    wrTa[:16, :64] = al * Wr.T
    wrTa[16:, 64:] = al * Wr.T
    wrTa = wrTa[:, inv]                     # permute output channels
    bgg = np.concatenate([bg, bg]).astype(f)
    bgp = bgg[inv % 64].reshape(128, 1)
    bthp = np.concatenate([bth, bth]).astype(f).reshape(32, 1)
    bphp = np.concatenate([bph, bph]).astype(f).reshape(32, 1)
    abr = (al * np.concatenate([br, br])).astype(f)
    abrp = abr[inv % 64].reshape(128, 1)
    S = (A + PA).astype(f)
    # strepA[p, u*26+v] = S[u,v] + alpha*br[inv(p)] for v<25; v=25 -> 0
    sU = np.zeros((V, V1), f)
    sU[:, :V] = S
    strepA = np.tile(sU.reshape(1, -1), (128, 1)).astype(f)
    mask = (np.arange(V * V1) % V1 < V).astype(f).reshape(1, -1)
    strepA = strepA + abrp @ mask
    cA = np.zeros((128, 654), f)
    cA[:, 0:1] = bgp
    cA[0:32, 1:2] = bthp
    cA[0:32, 2:3] = bphp
    cA[:, 4:654] = strepA
    bf16 = __import__("ml_dtypes").bfloat16
    cB = np.zeros((128, 448), f)
    cB[:, 0:128] = wgT
    cB[:, 128:160] = wthT
    cB[:, 160:192] = wphT
    cB[0:32, 192:320] = wrTa
    cB[:, 320:448] = np.eye(128, dtype=f)
    return dict(constsA=cA, constsB=cB.astype(bf16))


def kernel(**inputs):
    x = np.asarray(inputs["x"], np.float32)
    params = _host_params(
        np.asarray(inputs["A"], np.float32), np.asarray(inputs["PA"], np.float32),
        np.asarray(inputs["alpha"], np.float32), np.asarray(inputs["Wg"], np.float32),
        np.asarray(inputs["bg"], np.float32), np.asarray(inputs["Wth"], np.float32),
        np.asarray(inputs["bth"], np.float32), np.asarray(inputs["Wph"], np.float32),
        np.asarray(inputs["bph"], np.float32), np.asarray(inputs["Wr"], np.float32),
        np.asarray(inputs["br"], np.float32))

    if "nc" not in _cache:
        _cache["nc"] = _build_nc()
    nc = _cache["nc"]

    # upload x v-major: [NSH, C_IN, (v,t)]
    xv = np.ascontiguousarray(x.transpose(0, 1, 3, 2)).reshape(N, C_IN, TV)
    in_maps = []
    for i in range(NCORES):
        m = {"xs": xv[i * NSH:(i + 1) * NSH]}
        m.update(params)
        in_maps.append(m)

    res = run_bass_kernel_spmd(nc, in_maps, list(range(NCORES)),
                               **_cache.get("run_kwargs", {}))
    # device emits [NSH, T, (b16,cb4,u25)] bf16 per sample (c = 4b+cb)
    out = np.concatenate([np.asarray(res.results[i]["ys"]) for i in range(NCORES)],
                         axis=0)
    out = out.reshape(N, T, 16, 4, V).transpose(0, 2, 3, 1, 4).reshape(
        N, C_OUT, T, V)
    _cache["last_results"] = res
    return np.ascontiguousarray(out, dtype=np.float32)


if __name__ == "__main__":
    nc = _build_nc()
    print("build ok")
